# revision 14
# baseline (speedup 1.0000x reference)
"""DeepClusteringLoss Trainium2 kernel.

loss = mean_b || V_b V_b^T - Y_b Y_b^T ||_F^2 / T^2
     = mean_b ( ||V^T V||_F^2 - 2 ||V^T Y||_F^2 + ||Y^T Y||_F^2 ) / T^2

with V = row-L2-normalized embeddings.  B=16, T=16384, D=256, S=4.

Sharding: pure data parallel, 2 samples per core across 8 cores; each core
returns the un-normalized partial numerator sum for its 2 samples and the
host sums the 8 scalars and divides by B*T^2.

Per-core pipeline (per 2 MB "big tile" of 2048 t-rows = 16 sub-tiles of 128):
  - SWDGE DMA with f32->bf16 cast into SBUF [128, 16, 256]
  - row sum-of-squares: ACT Square+accum_out (12/16) + DVE fused
    tensor_tensor_reduce (4/16) -> n2 [128, 16]
  - 1/n = ACT Sqrt(DVE reciprocal(n2))
  - V = E * (1/n) on DVE tensor_scalar (bf16 4x mode), written into a
    [128, 16, 260] tile whose last 4 columns are the (raw) labels Y
  - PE accumulates over all 128 sub-tiles of a sample into PSUM:
      pA[128,260] = [G(d0,d0) | G(d0,d1) | Gvy(d0)]   (lhsT = V[:,0:128])
      pB[128,132] = [G(d1,d1) | Gvy(d1)]              (lhsT = V[:,128:256])
      pE[4,4]     = Gyy                               (lhsT = Y)
    (G(d1,d0) is skipped by symmetry; its squares count double.)
  - per-sample: DVE square-reduces of the PSUM blocks, combined as
      r_G00 + r_G11 + 2*(r_G01 - r_Gvy0 - r_Gvy1) + r_Gyy
  - cross-partition reduce via a tiny fp32 matmul against ones.
"""

import os
import sys

import numpy as np

sys.path.insert(0, "/opt/trn_rl_repo")

import concourse.bass as bass  # noqa: E402
import concourse.tile as tile  # noqa: E402
from concourse import mybir  # noqa: E402
from concourse.bass_utils import run_bass_kernel_spmd  # noqa: E402

B, T, D, S = 16, 16384, 256, 4
N_CORES = 8
SPC = B // N_CORES  # samples per core
P = 128  # partitions (t-rows per sub-tile)
NSUB = 16  # sub-tiles per big tile
BIG = T // (P * NSUB)  # big tiles per sample
NT = T // P  # label column groups per sample (128)
ACT_SCALES = 3  # sub-tiles per big tile whose V-scale runs on ACT (rest DVE)

F32 = mybir.dt.float32
BF16 = mybir.dt.bfloat16

_BUILT = None
LAST_RESULT = None  # BassKernelResults of the most recent run (for test.py)


def _build():
    FN = mybir.ActivationFunctionType
    OP = mybir.AluOpType

    nc = bass.Bass()
    emb = nc.dram_tensor("emb", [SPC * T, D], F32, kind="ExternalInput")
    lab = nc.dram_tensor("lab", [SPC, P, NT * S], F32, kind="ExternalInput")
    out = nc.dram_tensor("out", [1, 1], F32, kind="ExternalOutput")

    with tile.TileContext(nc) as tc:
        with (
            tc.tile_pool(name="eb", bufs=6) as eb_pool,
            tc.tile_pool(name="vy", bufs=4) as vy_pool,
            tc.tile_pool(name="yf", bufs=2) as yf_pool,
            tc.tile_pool(name="nrm", bufs=4) as nrm_pool,
            tc.tile_pool(name="scra", bufs=3) as scra_pool,
            tc.tile_pool(name="scrd", bufs=2) as scrd_pool,
            tc.tile_pool(name="ebm", bufs=2) as ebm_pool,
            tc.tile_pool(name="vym", bufs=2) as vym_pool,
            tc.tile_pool(name="esqm", bufs=2) as esqm_pool,
            tc.tile_pool(name="nrmm", bufs=2) as nrmm_pool,
            tc.tile_pool(name="red", bufs=2) as red_pool,
            tc.tile_pool(name="small", bufs=1) as small_pool,
            tc.tile_pool(name="psA", bufs=2, space="PSUM") as psA_pool,
            tc.tile_pool(name="psB", bufs=2, space="PSUM") as psB_pool,
            tc.tile_pool(name="psE", bufs=2, space="PSUM") as psE_pool,
            tc.tile_pool(name="psF", bufs=1, space="PSUM") as psF_pool,
        ):
            ones = small_pool.tile([P, 1], F32)
            nc.vector.memset(ones, 1.0)
            loss_parts = small_pool.tile([P, SPC], F32)

            for s in range(SPC):
                yf = yf_pool.tile([P, NT * S], F32)
                nc.sync.dma_start(out=yf[:], in_=lab[s])
                yv = yf[:].rearrange("p (n u) -> p n u", u=S)  # [128, 128, 4]

                pA = psA_pool.tile([P, D + S], F32)  # [G00 | G01 | Gvy0]
                pB = psB_pool.tile([P, D - P + S], F32)  # [G11 | Gvy1]
                pE = psE_pool.tile([S, S], F32)  # Gyy

                # Chunk plan: 2 MB tiles in steady state; the LAST sample
                # tapers its final tiles (16 -> 8 -> 4 sub-tiles) so the
                # kernel-tail dependency chain (square -> tree -> scales ->
                # matmuls) is short after the final DMA completes.
                if s == SPC - 1:
                    plan = [(g * 16, 16) for g in range(6)] + [
                        (96, 8), (104, 8),
                        (112, 4), (116, 4), (120, 4), (124, 4),
                    ]
                else:
                    plan = [(g * 16, 16) for g in range(BIG)]

                for ci, (n0, nsub) in enumerate(plan):
                    row0 = s * T + n0 * P
                    esrc = emb[row0 : row0 + P * nsub, :].rearrange(
                        "(n p) d -> p n d", p=P
                    )
                    if nsub == NSUB:
                        ebf = eb_pool.tile([P, nsub, D], BF16, tag="ebf")
                        esq = scra_pool.tile([P, nsub, D], BF16, tag="esq")
                        vy = vy_pool.tile([P, nsub, D + S], BF16, tag="vy")
                        n2 = nrm_pool.tile([P, nsub], F32, tag="n2")
                        invn2 = nrm_pool.tile([P, nsub], F32, tag="invn2")
                        invn = nrm_pool.tile([P, nsub], F32, tag="invn")
                        act_scales = ACT_SCALES
                    else:
                        ebf = ebm_pool.tile([P, nsub, D], BF16, tag=f"eb{nsub}")
                        esq = esqm_pool.tile([P, nsub, D], BF16, tag=f"es{nsub}")
                        vy = vym_pool.tile([P, nsub, D + S], BF16, tag=f"vy{nsub}")
                        n2 = nrmm_pool.tile([P, nsub], F32, tag=f"n2{nsub}")
                        invn2 = nrmm_pool.tile([P, nsub], F32, tag=f"iv2{nsub}")
                        invn = nrmm_pool.tile([P, nsub], F32, tag=f"iv{nsub}")
                        act_scales = 1 if nsub >= 8 else 0

                    nc.gpsimd.dma_start(out=ebf[:], in_=esrc)  # f32 -> bf16

                    # Row sums-of-squares: one big ACT Square, then an
                    # in-place pairwise-add tree + short segmented reduce on
                    # DVE (tensor_reduce is 1x-mode; the bf16 adds run 2x).
                    nc.scalar.square(out=esq[:], in_=ebf[:])
                    for L in (128, 64, 32):
                        nc.vector.tensor_add(
                            esq[:, :, 0:L], esq[:, :, 0:L], esq[:, :, L : 2 * L]
                        )
                    nc.vector.tensor_reduce(
                        out=n2[:],
                        in_=esq[:, :, 0:32],
                        axis=mybir.AxisListType.X,
                        op=OP.add,
                    )
                    nc.vector.reciprocal(invn2[:], n2[:])
                    nc.scalar.sqrt(invn[:], invn2[:])

                    for nl in range(nsub):
                        if nl < act_scales:
                            nc.scalar.mul(
                                out=vy[:, nl, 0:D],
                                in_=ebf[:, nl, :],
                                mul=invn[:, nl : nl + 1],
                            )
                        else:
                            nc.vector.tensor_scalar_mul(
                                out=vy[:, nl, 0:D],
                                in0=ebf[:, nl, :],
                                scalar1=invn[:, nl : nl + 1],
                            )
                    nc.vector.tensor_copy(
                        out=vy[:, :, D : D + S],
                        in_=yv[:, n0 : n0 + nsub, :],
                    )

                    for nl in range(nsub):
                        first = ci == 0 and nl == 0
                        last = ci == len(plan) - 1 and nl == nsub - 1
                        nc.tensor.matmul(
                            pA[:], vy[:, nl, 0:P], vy[:, nl, :],
                            start=first, stop=last,
                        )
                        nc.tensor.matmul(
                            pB[:], vy[:, nl, P:D], vy[:, nl, P:],
                            start=first, stop=last,
                        )
                        nc.tensor.matmul(
                            pE[:], vy[:, nl, D:], vy[:, nl, D:],
                            start=first, stop=last,
                        )

                # --- per-sample epilogue: sum of squares of the Gram blocks ---
                rr = red_pool.tile([P, 6], F32, tag="rr")

                def sqred(src, col, rows=P):
                    # ACT reads one PSUM input; Square + accum_out gives the
                    # block's sum of squares per partition in one op.
                    scr = scra_pool.tile([P, D], F32, tag="scra_ep")
                    nc.scalar.activation(
                        out=scr[0:rows, 0 : src.shape[-1]],
                        in_=src,
                        func=FN.Square,
                        accum_out=rr[0:rows, col : col + 1],
                    )

                sqred(pA[:, 0:P], 0)  # ||G00||^2
                sqred(pA[:, P:D], 1)  # ||G01||^2
                sqred(pA[:, D : D + S], 2)  # ||Gvy0||^2
                sqred(pB[:, 0:P], 3)  # ||G11||^2
                sqred(pB[:, P : P + S], 4)  # ||Gvy1||^2
                sqred(pE[:], 5, rows=S)  # ||Gyy||^2

                t1 = red_pool.tile([P, 1], F32, tag="t1")
                # t1 = r_Gvy0 + r_Gvy1
                nc.vector.tensor_add(t1[:], rr[:, 2:3], rr[:, 4:5])
                # t1 = r_G01 - t1
                nc.vector.tensor_tensor(t1[:], rr[:, 1:2], t1[:], op=OP.subtract)
                t2 = red_pool.tile([P, 1], F32, tag="t2")
                # t2 = r_G00 + r_G11
                nc.vector.tensor_add(t2[:], rr[:, 0:1], rr[:, 3:4])
                # t2 += r_Gyy  (only partitions 0:4)
                nc.vector.tensor_add(t2[0:S, :], t2[0:S, :], rr[0:S, 5:6])
                # loss_parts[:, s] = t2 + 2 * t1
                nc.vector.scalar_tensor_tensor(
                    out=loss_parts[:, s : s + 1],
                    in0=t1[:],
                    scalar=2.0,
                    in1=t2[:],
                    op0=OP.mult,
                    op1=OP.add,
                )

            # --- final: scalar = sum over partitions and samples ---
            loss_vec = small_pool.tile([P, 1], F32)
            nc.vector.tensor_reduce(
                out=loss_vec[:],
                in_=loss_parts[:],
                axis=mybir.AxisListType.X,
                op=mybir.AluOpType.add,
            )
            pF = psF_pool.tile([1, 1], F32)
            nc.tensor.matmul(pF[:], loss_vec[:], ones[:], start=True, stop=True)
            fin = small_pool.tile([1, 1], F32)
            nc.vector.tensor_copy(out=fin[:], in_=pF[:])
            nc.sync.dma_start(out=out[:], in_=fin[:])

    return nc


def _legalize_waits(nc):
    """Split multi-wait instructions for walrus' ISA wait-slot limits.

    This walrus build accepts at most 1 sync wait per regular instruction
    (2 per EventSemaphore), but Tile emits up to 3 (e.g. the kernel-tail
    Drain).  Hoist overflow waits onto EventSemaphore instructions inserted
    just before the over-subscribed instruction on the same engine queue.
    """
    for f in nc.m.functions:
        for blk in f.blocks:
            new_insts = []
            changed = False
            for inst in blk.instructions:
                si = inst.sync_info
                waits = list(si.on_wait) if si is not None and si.on_wait else []
                cap = 2 if isinstance(inst, mybir.InstEventSemaphore) else 1
                if len(waits) > cap:
                    changed = True
                    rest = waits[cap:]
                    for i in range(0, len(rest), 2):
                        ev = mybir.InstEventSemaphore(
                            name=f"{inst.name}-lw{i}",
                            ins=[],
                            outs=[],
                            engine=inst.engine,
                            sync_info=mybir.SyncInfo(
                                on_wait=rest[i : i + 2], on_update=[]
                            ),
                        )
                        new_insts.append(ev)
                    inst.sync_info = mybir.SyncInfo(
                        on_wait=waits[:cap],
                        on_update=list(si.on_update) if si.on_update else [],
                    )
                new_insts.append(inst)
            if changed:
                blk.instructions = new_insts


def _get_built():
    global _BUILT
    if _BUILT is None:
        nc = _build()
        _legalize_waits(nc)
        _BUILT = nc
    return _BUILT


def kernel(embeddings: np.ndarray, labels: np.ndarray) -> np.ndarray:
    global LAST_RESULT
    nc = _get_built()

    emb_np = np.ascontiguousarray(np.asarray(embeddings, dtype=np.float32)).reshape(
        B, T, D
    )
    lab_np = np.ascontiguousarray(np.asarray(labels, dtype=np.float32)).reshape(
        B, T, S
    )
    # [b, t, u] -> [b, p, n, u] with t = n*128 + p, flattened to [b, 128, NT*S]
    lab_pre = (
        lab_np.reshape(B, NT, P, S).transpose(0, 2, 1, 3).reshape(B, P, NT * S)
    )

    in_maps = []
    for i in range(N_CORES):
        in_maps.append(
            {
                "emb": np.ascontiguousarray(
                    emb_np[i * SPC : (i + 1) * SPC].reshape(SPC * T, D)
                ),
                "lab": np.ascontiguousarray(lab_pre[i * SPC : (i + 1) * SPC]),
            }
        )

    trace = os.environ.get("KERNEL_TRACE", "0") == "1"
    res = run_bass_kernel_spmd(
        nc, in_maps, core_ids=list(range(N_CORES)), trace=trace
    )
    LAST_RESULT = res

    total = 0.0
    for r in res.results:
        total += float(r["out"][0, 0])
    loss = total / (B * float(T) * float(T))
    return np.float32(loss)


# revision 15
# speedup vs baseline: 1.0503x; 1.0503x over previous
"""DeepClusteringLoss Trainium2 kernel.

loss = mean_b || V_b V_b^T - Y_b Y_b^T ||_F^2 / T^2
     = mean_b ( ||V^T V||_F^2 - 2 ||V^T Y||_F^2 + ||Y^T Y||_F^2 ) / T^2

with V = row-L2-normalized embeddings.  B=16, T=16384, D=256, S=4.

Sharding: pure data parallel, 2 samples per core across 8 cores; each core
returns the un-normalized partial numerator sum for its 2 samples and the
host sums the 8 scalars and divides by B*T^2.

Per-core pipeline (per 2 MB "big tile" of 2048 t-rows = 16 sub-tiles of 128):
  - SWDGE DMA with f32->bf16 cast into SBUF [128, 16, 256]
  - row sum-of-squares: ACT Square+accum_out (12/16) + DVE fused
    tensor_tensor_reduce (4/16) -> n2 [128, 16]
  - 1/n = ACT Sqrt(DVE reciprocal(n2))
  - V = E * (1/n) on DVE tensor_scalar (bf16 4x mode), written into a
    [128, 16, 260] tile whose last 4 columns are the (raw) labels Y
  - PE accumulates over all 128 sub-tiles of a sample into PSUM:
      pA[128,260] = [G(d0,d0) | G(d0,d1) | Gvy(d0)]   (lhsT = V[:,0:128])
      pB[128,132] = [G(d1,d1) | Gvy(d1)]              (lhsT = V[:,128:256])
      pE[4,4]     = Gyy                               (lhsT = Y)
    (G(d1,d0) is skipped by symmetry; its squares count double.)
  - per-sample: DVE square-reduces of the PSUM blocks, combined as
      r_G00 + r_G11 + 2*(r_G01 - r_Gvy0 - r_Gvy1) + r_Gyy
  - cross-partition reduce via a tiny fp32 matmul against ones.
"""

import os
import sys

import numpy as np

sys.path.insert(0, "/opt/trn_rl_repo")

import concourse.bass as bass  # noqa: E402
import concourse.tile as tile  # noqa: E402
from concourse import mybir  # noqa: E402
from concourse.bass_utils import run_bass_kernel_spmd  # noqa: E402

B, T, D, S = 16, 16384, 256, 4
N_CORES = 8
SPC = B // N_CORES  # samples per core
P = 128  # partitions (t-rows per sub-tile)
NSUB = 16  # sub-tiles per big tile
BIG = T // (P * NSUB)  # big tiles per sample
NT = T // P  # label column groups per sample (128)
ACT_SCALES = 3  # sub-tiles per big tile whose V-scale runs on ACT (rest DVE)

F32 = mybir.dt.float32
BF16 = mybir.dt.bfloat16

_BUILT = None
LAST_RESULT = None  # BassKernelResults of the most recent run (for test.py)


def _build():
    FN = mybir.ActivationFunctionType
    OP = mybir.AluOpType

    nc = bass.Bass()
    emb = nc.dram_tensor("emb", [SPC * T, D], F32, kind="ExternalInput")
    lab = nc.dram_tensor("lab", [SPC, P, NT * S], F32, kind="ExternalInput")
    out = nc.dram_tensor("out", [1, 1], F32, kind="ExternalOutput")

    with tile.TileContext(nc) as tc:
        with (
            tc.tile_pool(name="eb", bufs=6) as eb_pool,
            tc.tile_pool(name="vy", bufs=4) as vy_pool,
            tc.tile_pool(name="yf", bufs=2) as yf_pool,
            tc.tile_pool(name="nrm", bufs=4) as nrm_pool,
            tc.tile_pool(name="scra", bufs=3) as scra_pool,
            tc.tile_pool(name="scrd", bufs=2) as scrd_pool,
            tc.tile_pool(name="ebm", bufs=2) as ebm_pool,
            tc.tile_pool(name="vym", bufs=2) as vym_pool,
            tc.tile_pool(name="esqm", bufs=2) as esqm_pool,
            tc.tile_pool(name="nrmm", bufs=2) as nrmm_pool,
            tc.tile_pool(name="red", bufs=2) as red_pool,
            tc.tile_pool(name="small", bufs=1) as small_pool,
            tc.tile_pool(name="psA", bufs=2, space="PSUM") as psA_pool,
            tc.tile_pool(name="psB", bufs=2, space="PSUM") as psB_pool,
            tc.tile_pool(name="psE", bufs=2, space="PSUM") as psE_pool,
            tc.tile_pool(name="psF", bufs=1, space="PSUM") as psF_pool,
        ):
            ones = small_pool.tile([P, 1], F32)
            nc.vector.memset(ones, 1.0)
            loss_parts = small_pool.tile([P, SPC], F32)

            for s in range(SPC):
                yf = yf_pool.tile([P, NT * S], F32)
                nc.sync.dma_start(out=yf[:], in_=lab[s])
                yv = yf[:].rearrange("p (n u) -> p n u", u=S)  # [128, 128, 4]

                pA = psA_pool.tile([P, D + S], F32)  # [G00 | G01 | Gvy0]
                pB = psB_pool.tile([P, D - P + S], F32)  # [G11 | Gvy1]
                pE = psE_pool.tile([S, S], F32)  # Gyy

                # Chunk plan: 2 MB tiles in steady state; the LAST sample
                # tapers its final tiles (16 -> 8 -> 4 sub-tiles) so the
                # kernel-tail dependency chain (square -> tree -> scales ->
                # matmuls) is short after the final DMA completes.
                if s == SPC - 1:
                    plan = [(g * 16, 16) for g in range(7)] + [
                        (112, 8), (120, 8),
                    ]
                else:
                    plan = [(g * 16, 16) for g in range(BIG)]

                for ci, (n0, nsub) in enumerate(plan):
                    row0 = s * T + n0 * P
                    esrc = emb[row0 : row0 + P * nsub, :].rearrange(
                        "(n p) d -> p n d", p=P
                    )
                    if nsub == NSUB:
                        ebf = eb_pool.tile([P, nsub, D], BF16, tag="ebf")
                        esq = scra_pool.tile([P, nsub, D], BF16, tag="esq")
                        vy = vy_pool.tile([P, nsub, D + S], BF16, tag="vy")
                        n2 = nrm_pool.tile([P, nsub], F32, tag="n2")
                        invn2 = nrm_pool.tile([P, nsub], F32, tag="invn2")
                        invn = nrm_pool.tile([P, nsub], F32, tag="invn")
                        act_scales = ACT_SCALES
                    else:
                        ebf = ebm_pool.tile([P, nsub, D], BF16, tag=f"eb{nsub}")
                        esq = esqm_pool.tile([P, nsub, D], BF16, tag=f"es{nsub}")
                        vy = vym_pool.tile([P, nsub, D + S], BF16, tag=f"vy{nsub}")
                        n2 = nrmm_pool.tile([P, nsub], F32, tag=f"n2{nsub}")
                        invn2 = nrmm_pool.tile([P, nsub], F32, tag=f"iv2{nsub}")
                        invn = nrmm_pool.tile([P, nsub], F32, tag=f"iv{nsub}")
                        act_scales = 1 if nsub >= 8 else 0

                    nc.gpsimd.dma_start(out=ebf[:], in_=esrc)  # f32 -> bf16

                    # Row sums-of-squares: one big ACT Square, then an
                    # in-place pairwise-add tree + short segmented reduce on
                    # DVE (tensor_reduce is 1x-mode; the bf16 adds run 2x).
                    nc.scalar.square(out=esq[:], in_=ebf[:])
                    for L in (128, 64, 32):
                        nc.vector.tensor_add(
                            esq[:, :, 0:L], esq[:, :, 0:L], esq[:, :, L : 2 * L]
                        )
                    nc.vector.tensor_reduce(
                        out=n2[:],
                        in_=esq[:, :, 0:32],
                        axis=mybir.AxisListType.X,
                        op=OP.add,
                    )
                    nc.vector.reciprocal(invn2[:], n2[:])
                    nc.scalar.sqrt(invn[:], invn2[:])

                    for nl in range(nsub):
                        if nl < act_scales:
                            nc.scalar.mul(
                                out=vy[:, nl, 0:D],
                                in_=ebf[:, nl, :],
                                mul=invn[:, nl : nl + 1],
                            )
                        else:
                            nc.vector.tensor_scalar_mul(
                                out=vy[:, nl, 0:D],
                                in0=ebf[:, nl, :],
                                scalar1=invn[:, nl : nl + 1],
                            )
                    nc.vector.tensor_copy(
                        out=vy[:, :, D : D + S],
                        in_=yv[:, n0 : n0 + nsub, :],
                    )

                    for nl in range(nsub):
                        first = ci == 0 and nl == 0
                        last = ci == len(plan) - 1 and nl == nsub - 1
                        nc.tensor.matmul(
                            pA[:], vy[:, nl, 0:P], vy[:, nl, :],
                            start=first, stop=last,
                        )
                        nc.tensor.matmul(
                            pB[:], vy[:, nl, P:D], vy[:, nl, P:],
                            start=first, stop=last,
                        )
                        nc.tensor.matmul(
                            pE[:], vy[:, nl, D:], vy[:, nl, D:],
                            start=first, stop=last,
                        )

                # --- per-sample epilogue: sum of squares of the Gram blocks ---
                rr = red_pool.tile([P, 6], F32, tag="rr")

                def sqred(src, col, rows=P):
                    # ACT reads one PSUM input; Square + accum_out gives the
                    # block's sum of squares per partition in one op.
                    scr = scra_pool.tile([P, D], F32, tag="scra_ep")
                    nc.scalar.activation(
                        out=scr[0:rows, 0 : src.shape[-1]],
                        in_=src,
                        func=FN.Square,
                        accum_out=rr[0:rows, col : col + 1],
                    )

                sqred(pA[:, 0:P], 0)  # ||G00||^2
                sqred(pA[:, P:D], 1)  # ||G01||^2
                sqred(pA[:, D : D + S], 2)  # ||Gvy0||^2
                sqred(pB[:, 0:P], 3)  # ||G11||^2
                sqred(pB[:, P : P + S], 4)  # ||Gvy1||^2
                sqred(pE[:], 5, rows=S)  # ||Gyy||^2

                t1 = red_pool.tile([P, 1], F32, tag="t1")
                # t1 = r_Gvy0 + r_Gvy1
                nc.vector.tensor_add(t1[:], rr[:, 2:3], rr[:, 4:5])
                # t1 = r_G01 - t1
                nc.vector.tensor_tensor(t1[:], rr[:, 1:2], t1[:], op=OP.subtract)
                t2 = red_pool.tile([P, 1], F32, tag="t2")
                # t2 = r_G00 + r_G11
                nc.vector.tensor_add(t2[:], rr[:, 0:1], rr[:, 3:4])
                # t2 += r_Gyy  (only partitions 0:4)
                nc.vector.tensor_add(t2[0:S, :], t2[0:S, :], rr[0:S, 5:6])
                # loss_parts[:, s] = t2 + 2 * t1
                nc.vector.scalar_tensor_tensor(
                    out=loss_parts[:, s : s + 1],
                    in0=t1[:],
                    scalar=2.0,
                    in1=t2[:],
                    op0=OP.mult,
                    op1=OP.add,
                )

            # --- final: scalar = sum over partitions and samples ---
            loss_vec = small_pool.tile([P, 1], F32)
            nc.vector.tensor_reduce(
                out=loss_vec[:],
                in_=loss_parts[:],
                axis=mybir.AxisListType.X,
                op=mybir.AluOpType.add,
            )
            pF = psF_pool.tile([1, 1], F32)
            nc.tensor.matmul(pF[:], loss_vec[:], ones[:], start=True, stop=True)
            fin = small_pool.tile([1, 1], F32)
            nc.vector.tensor_copy(out=fin[:], in_=pF[:])
            nc.sync.dma_start(out=out[:], in_=fin[:])

    return nc


def _legalize_waits(nc):
    """Split multi-wait instructions for walrus' ISA wait-slot limits.

    This walrus build accepts at most 1 sync wait per regular instruction
    (2 per EventSemaphore), but Tile emits up to 3 (e.g. the kernel-tail
    Drain).  Hoist overflow waits onto EventSemaphore instructions inserted
    just before the over-subscribed instruction on the same engine queue.
    """
    for f in nc.m.functions:
        for blk in f.blocks:
            new_insts = []
            changed = False
            for inst in blk.instructions:
                si = inst.sync_info
                waits = list(si.on_wait) if si is not None and si.on_wait else []
                cap = 2 if isinstance(inst, mybir.InstEventSemaphore) else 1
                if len(waits) > cap:
                    changed = True
                    rest = waits[cap:]
                    for i in range(0, len(rest), 2):
                        ev = mybir.InstEventSemaphore(
                            name=f"{inst.name}-lw{i}",
                            ins=[],
                            outs=[],
                            engine=inst.engine,
                            sync_info=mybir.SyncInfo(
                                on_wait=rest[i : i + 2], on_update=[]
                            ),
                        )
                        new_insts.append(ev)
                    inst.sync_info = mybir.SyncInfo(
                        on_wait=waits[:cap],
                        on_update=list(si.on_update) if si.on_update else [],
                    )
                new_insts.append(inst)
            if changed:
                blk.instructions = new_insts


def _get_built():
    global _BUILT
    if _BUILT is None:
        nc = _build()
        _legalize_waits(nc)
        _BUILT = nc
    return _BUILT


def kernel(embeddings: np.ndarray, labels: np.ndarray) -> np.ndarray:
    global LAST_RESULT
    nc = _get_built()

    emb_np = np.ascontiguousarray(np.asarray(embeddings, dtype=np.float32)).reshape(
        B, T, D
    )
    lab_np = np.ascontiguousarray(np.asarray(labels, dtype=np.float32)).reshape(
        B, T, S
    )
    # [b, t, u] -> [b, p, n, u] with t = n*128 + p, flattened to [b, 128, NT*S]
    lab_pre = (
        lab_np.reshape(B, NT, P, S).transpose(0, 2, 1, 3).reshape(B, P, NT * S)
    )

    in_maps = []
    for i in range(N_CORES):
        in_maps.append(
            {
                "emb": np.ascontiguousarray(
                    emb_np[i * SPC : (i + 1) * SPC].reshape(SPC * T, D)
                ),
                "lab": np.ascontiguousarray(lab_pre[i * SPC : (i + 1) * SPC]),
            }
        )

    trace = os.environ.get("KERNEL_TRACE", "0") == "1"
    res = run_bass_kernel_spmd(
        nc, in_maps, core_ids=list(range(N_CORES)), trace=trace
    )
    LAST_RESULT = res

    total = 0.0
    for r in res.results:
        total += float(r["out"][0, 0])
    loss = total / (B * float(T) * float(T))
    return np.float32(loss)


# revision 17
# speedup vs baseline: 1.0571x; 1.0066x over previous
"""DeepClusteringLoss Trainium2 kernel.

loss = mean_b || V_b V_b^T - Y_b Y_b^T ||_F^2 / T^2
     = mean_b ( ||V^T V||_F^2 - 2 ||V^T Y||_F^2 + ||Y^T Y||_F^2 ) / T^2

with V = row-L2-normalized embeddings.  B=16, T=16384, D=256, S=4.

Sharding: pure data parallel, 2 samples per core across 8 cores; each core
returns the un-normalized partial numerator sum for its 2 samples and the
host sums the 8 scalars and divides by B*T^2.

Per-core pipeline (per 2 MB "big tile" of 2048 t-rows = 16 sub-tiles of 128):
  - SWDGE DMA with f32->bf16 cast into SBUF [128, 16, 256]
  - row sum-of-squares: ACT Square+accum_out (12/16) + DVE fused
    tensor_tensor_reduce (4/16) -> n2 [128, 16]
  - 1/n = ACT Sqrt(DVE reciprocal(n2))
  - V = E * (1/n) on DVE tensor_scalar (bf16 4x mode), written into a
    [128, 16, 260] tile whose last 4 columns are the (raw) labels Y
  - PE accumulates over all 128 sub-tiles of a sample into PSUM:
      pA[128,260] = [G(d0,d0) | G(d0,d1) | Gvy(d0)]   (lhsT = V[:,0:128])
      pB[128,132] = [G(d1,d1) | Gvy(d1)]              (lhsT = V[:,128:256])
      pE[4,4]     = Gyy                               (lhsT = Y)
    (G(d1,d0) is skipped by symmetry; its squares count double.)
  - per-sample: DVE square-reduces of the PSUM blocks, combined as
      r_G00 + r_G11 + 2*(r_G01 - r_Gvy0 - r_Gvy1) + r_Gyy
  - cross-partition reduce via a tiny fp32 matmul against ones.
"""

import os
import sys

import numpy as np

sys.path.insert(0, "/opt/trn_rl_repo")

import concourse.bass as bass  # noqa: E402
import concourse.tile as tile  # noqa: E402
from concourse import mybir  # noqa: E402
from concourse.bass_utils import run_bass_kernel_spmd  # noqa: E402

B, T, D, S = 16, 16384, 256, 4
N_CORES = 8
SPC = B // N_CORES  # samples per core
P = 128  # partitions (t-rows per sub-tile)
NSUB = 16  # sub-tiles per big tile
BIG = T // (P * NSUB)  # big tiles per sample
NT = T // P  # label column groups per sample (128)
ACT_SCALES = 3  # sub-tiles per big tile whose V-scale runs on ACT (rest DVE)

F32 = mybir.dt.float32
BF16 = mybir.dt.bfloat16

_BUILT = None
LAST_RESULT = None  # BassKernelResults of the most recent run (for test.py)


def _build():
    FN = mybir.ActivationFunctionType
    OP = mybir.AluOpType

    nc = bass.Bass()
    emb = nc.dram_tensor("emb", [SPC * T, D], F32, kind="ExternalInput")
    lab = nc.dram_tensor("lab", [SPC, P, NT * S], F32, kind="ExternalInput")
    out = nc.dram_tensor("out", [1, 1], F32, kind="ExternalOutput")

    with tile.TileContext(nc) as tc:
        with (
            tc.tile_pool(name="eb", bufs=8) as eb_pool,
            tc.tile_pool(name="vy", bufs=6) as vy_pool,
            tc.tile_pool(name="yf", bufs=2) as yf_pool,
            tc.tile_pool(name="nrm", bufs=6) as nrm_pool,
            tc.tile_pool(name="scra", bufs=4) as scra_pool,
            tc.tile_pool(name="scrd", bufs=2) as scrd_pool,
            tc.tile_pool(name="ebm", bufs=2) as ebm_pool,
            tc.tile_pool(name="vym", bufs=2) as vym_pool,
            tc.tile_pool(name="esqm", bufs=2) as esqm_pool,
            tc.tile_pool(name="nrmm", bufs=2) as nrmm_pool,
            tc.tile_pool(name="red", bufs=2) as red_pool,
            tc.tile_pool(name="small", bufs=1) as small_pool,
            tc.tile_pool(name="psA", bufs=2, space="PSUM") as psA_pool,
            tc.tile_pool(name="psB", bufs=2, space="PSUM") as psB_pool,
            tc.tile_pool(name="psE", bufs=2, space="PSUM") as psE_pool,
            tc.tile_pool(name="psF", bufs=1, space="PSUM") as psF_pool,
        ):
            ones = small_pool.tile([P, 1], F32)
            nc.vector.memset(ones, 1.0)
            loss_parts = small_pool.tile([P, SPC], F32)

            for s in range(SPC):
                yf = yf_pool.tile([P, NT * S], F32)
                nc.sync.dma_start(out=yf[:], in_=lab[s])
                yv = yf[:].rearrange("p (n u) -> p n u", u=S)  # [128, 128, 4]

                pA = psA_pool.tile([P, D + S], F32)  # [G00 | G01 | Gvy0]
                pB = psB_pool.tile([P, D - P + S], F32)  # [G11 | Gvy1]
                pE = psE_pool.tile([S, S], F32)  # Gyy

                # Chunk plan: 2 MB tiles in steady state; the LAST sample
                # tapers its final tiles (16 -> 8 -> 4 sub-tiles) so the
                # kernel-tail dependency chain (square -> tree -> scales ->
                # matmuls) is short after the final DMA completes.
                plan = [(g * 16, 16) for g in range(BIG)]

                for ci, (n0, nsub) in enumerate(plan):
                    row0 = s * T + n0 * P
                    esrc = emb[row0 : row0 + P * nsub, :].rearrange(
                        "(n p) d -> p n d", p=P
                    )
                    if nsub == NSUB:
                        ebf = eb_pool.tile([P, nsub, D], BF16, tag="ebf")
                        esq = scra_pool.tile([P, nsub, D], BF16, tag="esq")
                        vy = vy_pool.tile([P, nsub, D + S], BF16, tag="vy")
                        n2 = nrm_pool.tile([P, nsub], F32, tag="n2")
                        invn2 = nrm_pool.tile([P, nsub], F32, tag="invn2")
                        invn = nrm_pool.tile([P, nsub], F32, tag="invn")
                        act_scales = ACT_SCALES
                    else:
                        ebf = ebm_pool.tile([P, nsub, D], BF16, tag=f"eb{nsub}")
                        esq = esqm_pool.tile([P, nsub, D], BF16, tag=f"es{nsub}")
                        vy = vym_pool.tile([P, nsub, D + S], BF16, tag=f"vy{nsub}")
                        n2 = nrmm_pool.tile([P, nsub], F32, tag=f"n2{nsub}")
                        invn2 = nrmm_pool.tile([P, nsub], F32, tag=f"iv2{nsub}")
                        invn = nrmm_pool.tile([P, nsub], F32, tag=f"iv{nsub}")
                        act_scales = 1 if nsub >= 8 else 0

                    nc.gpsimd.dma_start(out=ebf[:], in_=esrc)  # f32 -> bf16

                    # Row sums-of-squares: one big ACT Square, then an
                    # in-place pairwise-add tree + short segmented reduce on
                    # DVE (tensor_reduce is 1x-mode; the bf16 adds run 2x).
                    nc.scalar.square(out=esq[:], in_=ebf[:])
                    for L in (128, 64, 32):
                        nc.vector.tensor_add(
                            esq[:, :, 0:L], esq[:, :, 0:L], esq[:, :, L : 2 * L]
                        )
                    nc.vector.tensor_reduce(
                        out=n2[:],
                        in_=esq[:, :, 0:32],
                        axis=mybir.AxisListType.X,
                        op=OP.add,
                    )
                    nc.vector.reciprocal(invn2[:], n2[:])
                    nc.scalar.sqrt(invn[:], invn2[:])

                    for nl in range(nsub):
                        if nl < act_scales:
                            nc.scalar.mul(
                                out=vy[:, nl, 0:D],
                                in_=ebf[:, nl, :],
                                mul=invn[:, nl : nl + 1],
                            )
                        else:
                            nc.vector.tensor_scalar_mul(
                                out=vy[:, nl, 0:D],
                                in0=ebf[:, nl, :],
                                scalar1=invn[:, nl : nl + 1],
                            )
                    nc.gpsimd.tensor_copy(
                        out=vy[:, :, D : D + S],
                        in_=yv[:, n0 : n0 + nsub, :],
                    )

                    for nl in range(nsub):
                        first = ci == 0 and nl == 0
                        last = ci == len(plan) - 1 and nl == nsub - 1
                        nc.tensor.matmul(
                            pA[:], vy[:, nl, 0:P], vy[:, nl, :],
                            start=first, stop=last,
                        )
                        nc.tensor.matmul(
                            pB[:], vy[:, nl, P:D], vy[:, nl, P:],
                            start=first, stop=last,
                        )
                        nc.tensor.matmul(
                            pE[:], vy[:, nl, D:], vy[:, nl, D:],
                            start=first, stop=last,
                        )

                # --- per-sample epilogue: sum of squares of the Gram blocks ---
                rr = red_pool.tile([P, 6], F32, tag="rr")

                def sqred(src, col, rows=P):
                    # ACT reads one PSUM input; Square + accum_out gives the
                    # block's sum of squares per partition in one op.
                    scr = scra_pool.tile([P, D], F32, tag="scra_ep")
                    nc.scalar.activation(
                        out=scr[0:rows, 0 : src.shape[-1]],
                        in_=src,
                        func=FN.Square,
                        accum_out=rr[0:rows, col : col + 1],
                    )

                sqred(pA[:, 0:P], 0)  # ||G00||^2
                sqred(pA[:, P:D], 1)  # ||G01||^2
                sqred(pA[:, D : D + S], 2)  # ||Gvy0||^2
                sqred(pB[:, 0:P], 3)  # ||G11||^2
                sqred(pB[:, P : P + S], 4)  # ||Gvy1||^2
                sqred(pE[:], 5, rows=S)  # ||Gyy||^2

                t1 = red_pool.tile([P, 1], F32, tag="t1")
                # t1 = r_Gvy0 + r_Gvy1
                nc.vector.tensor_add(t1[:], rr[:, 2:3], rr[:, 4:5])
                # t1 = r_G01 - t1
                nc.vector.tensor_tensor(t1[:], rr[:, 1:2], t1[:], op=OP.subtract)
                t2 = red_pool.tile([P, 1], F32, tag="t2")
                # t2 = r_G00 + r_G11
                nc.vector.tensor_add(t2[:], rr[:, 0:1], rr[:, 3:4])
                # t2 += r_Gyy  (only partitions 0:4)
                nc.vector.tensor_add(t2[0:S, :], t2[0:S, :], rr[0:S, 5:6])
                # loss_parts[:, s] = t2 + 2 * t1
                nc.vector.scalar_tensor_tensor(
                    out=loss_parts[:, s : s + 1],
                    in0=t1[:],
                    scalar=2.0,
                    in1=t2[:],
                    op0=OP.mult,
                    op1=OP.add,
                )

            # --- final: scalar = sum over partitions and samples ---
            loss_vec = small_pool.tile([P, 1], F32)
            nc.vector.tensor_reduce(
                out=loss_vec[:],
                in_=loss_parts[:],
                axis=mybir.AxisListType.X,
                op=mybir.AluOpType.add,
            )
            pF = psF_pool.tile([1, 1], F32)
            nc.tensor.matmul(pF[:], loss_vec[:], ones[:], start=True, stop=True)
            fin = small_pool.tile([1, 1], F32)
            nc.vector.tensor_copy(out=fin[:], in_=pF[:])
            nc.sync.dma_start(out=out[:], in_=fin[:])

    return nc


def _legalize_waits(nc):
    """Split multi-wait instructions for walrus' ISA wait-slot limits.

    This walrus build accepts at most 1 sync wait per regular instruction
    (2 per EventSemaphore), but Tile emits up to 3 (e.g. the kernel-tail
    Drain).  Hoist overflow waits onto EventSemaphore instructions inserted
    just before the over-subscribed instruction on the same engine queue.
    """
    for f in nc.m.functions:
        for blk in f.blocks:
            new_insts = []
            changed = False
            for inst in blk.instructions:
                si = inst.sync_info
                waits = list(si.on_wait) if si is not None and si.on_wait else []
                cap = 2 if isinstance(inst, mybir.InstEventSemaphore) else 1
                if len(waits) > cap:
                    changed = True
                    rest = waits[cap:]
                    for i in range(0, len(rest), 2):
                        ev = mybir.InstEventSemaphore(
                            name=f"{inst.name}-lw{i}",
                            ins=[],
                            outs=[],
                            engine=inst.engine,
                            sync_info=mybir.SyncInfo(
                                on_wait=rest[i : i + 2], on_update=[]
                            ),
                        )
                        new_insts.append(ev)
                    inst.sync_info = mybir.SyncInfo(
                        on_wait=waits[:cap],
                        on_update=list(si.on_update) if si.on_update else [],
                    )
                new_insts.append(inst)
            if changed:
                blk.instructions = new_insts


def _get_built():
    global _BUILT
    if _BUILT is None:
        nc = _build()
        _legalize_waits(nc)
        _BUILT = nc
    return _BUILT


def kernel(embeddings: np.ndarray, labels: np.ndarray) -> np.ndarray:
    global LAST_RESULT
    nc = _get_built()

    emb_np = np.ascontiguousarray(np.asarray(embeddings, dtype=np.float32)).reshape(
        B, T, D
    )
    lab_np = np.ascontiguousarray(np.asarray(labels, dtype=np.float32)).reshape(
        B, T, S
    )
    # [b, t, u] -> [b, p, n, u] with t = n*128 + p, flattened to [b, 128, NT*S]
    lab_pre = (
        lab_np.reshape(B, NT, P, S).transpose(0, 2, 1, 3).reshape(B, P, NT * S)
    )

    in_maps = []
    for i in range(N_CORES):
        in_maps.append(
            {
                "emb": np.ascontiguousarray(
                    emb_np[i * SPC : (i + 1) * SPC].reshape(SPC * T, D)
                ),
                "lab": np.ascontiguousarray(lab_pre[i * SPC : (i + 1) * SPC]),
            }
        )

    trace = os.environ.get("KERNEL_TRACE", "0") == "1"
    res = run_bass_kernel_spmd(
        nc, in_maps, core_ids=list(range(N_CORES)), trace=trace
    )
    LAST_RESULT = res

    total = 0.0
    for r in res.results:
        total += float(r["out"][0, 0])
    loss = total / (B * float(T) * float(T))
    return np.float32(loss)
